# revision 20
# baseline (speedup 1.0000x reference)
"""Trainium2 Bass kernel for BidirectionalAttention.

Math (per batch b):
    xf = x[b].reshape(C, N)                    # C=256, N=4096
    q = Wq @ xf + bq ; k = Wk @ xf + bk        # [32, N]
    v = Wv @ xf + bv                           # [256, N]
    A = softmax_m(q^T k)                       # softmax over keys m
    out = v @ A^T ; y = x + gamma * out        # returned twice

Sharding: 8 cores = (batch b = core//2) x (query-half = core%2).
Attention is permutation-invariant over keys, so the host rotates each
core's image so its query half is always columns 0..2047 — one program
serves all cores.

On-core layout: scores are computed transposed (S^T[m, n]) so exp(S^T)
already has the contraction dim (m) on partitions for the second matmul
U^T[n, c] = sum_m E^T[m, n] * vT[m, c].  vT carries an appended ones
column so the same matmul chain yields the softmax denominator Z[n] for
free, and an appended K-row carries bv.  Normalization + gamma are a
per-partition scale on U^T; PE transposes back to [c, n] for the
residual add.  Score matmuls (K=32) are packed 4-up into the PE array
via tile_position row groups; exp runs over 2-bank PSUM tiles (FD=1024)
to halve ACT instruction overhead.
"""

import numpy as np

C = 256
C8 = 32
NPIX = 4096     # 64*64
NQ = 2048       # queries per core
B = 4
NCORES = 8
MT = NPIX // 128   # 32 key tiles
NCH = NQ // 512    # 4 query chunks per core
NG = MT // 4       # 8 groups of 4 key tiles

_cache = {}


def _build(dbg=False):
    import concourse.bacc as bacc
    import concourse.tile as tile
    from concourse import mybir
    from contextlib import ExitStack

    FP32 = mybir.dt.float32
    BF16 = mybir.dt.bfloat16
    AF = mybir.ActivationFunctionType

    nc = bacc.Bacc("TRN2", target_bir_lowering=False, debug=False)

    xf_d = nc.dram_tensor("xf", [C, NPIX], FP32, kind="ExternalInput")
    wq_d = nc.dram_tensor("wq", [C8, C], FP32, kind="ExternalInput")
    wk_d = nc.dram_tensor("wk", [C8, C], FP32, kind="ExternalInput")
    wv_d = nc.dram_tensor("wv", [C, C], FP32, kind="ExternalInput")
    bq_d = nc.dram_tensor("bq", [C8, 1], FP32, kind="ExternalInput")
    bk_d = nc.dram_tensor("bk", [C8, 1], FP32, kind="ExternalInput")
    bv_d = nc.dram_tensor("bv", [1, C], FP32, kind="ExternalInput")
    g_d = nc.dram_tensor("gamma", [1, 1], FP32, kind="ExternalInput")
    y_d = nc.dram_tensor("y", [C, NQ], FP32, kind="ExternalOutput")
    if dbg:
        qd_d = nc.dram_tensor("qdump", [128, NQ], BF16, kind="ExternalOutput")
        kd_d = nc.dram_tensor("kdump", [128, NG, 128], BF16,
                              kind="ExternalOutput")
        vtd_d = nc.dram_tensor("vtdump", [128, MT, C + 1], BF16,
                               kind="ExternalOutput")
        etd_d = nc.dram_tensor("etdump", [128, 2, 512], BF16,
                               kind="ExternalOutput")
        utd_d = nc.dram_tensor("utdump", [128, C + 1], FP32,
                               kind="ExternalOutput")

    with tile.TileContext(nc) as tc, ExitStack() as ctx:
        consts = ctx.enter_context(tc.tile_pool(name="consts", bufs=1))
        stage = ctx.enter_context(tc.tile_pool(name="stage", bufs=2))
        etp = ctx.enter_context(tc.tile_pool(name="et", bufs=4))
        yp = ctx.enter_context(tc.tile_pool(name="yp", bufs=2))
        small = ctx.enter_context(tc.tile_pool(name="small", bufs=8))
        # PSUM: tag "s" = 2 slots x 2 banks (scores), tag "ut" = 4 slots x
        # 1 bank (U^T accumulators + all setup/epilogue psums) = 8 banks.
        ps_s = ctx.enter_context(tc.tile_pool(name="ps_s", bufs=2, space="PSUM"))
        ps_u = ctx.enter_context(tc.tile_pool(name="ps_u", bufs=4, space="PSUM"))

        # ---- persistent SBUF tensors ----
        ident = consts.tile([128, 128], BF16)
        identf = consts.tile([128, 128], FP32)
        it = consts.tile([128, 128], mybir.dt.int32)
        nc.gpsimd.iota(it[:], pattern=[[-1, 128]], base=0, channel_multiplier=1)
        nc.vector.tensor_scalar(out=ident[:], in0=it[:], scalar1=0,
                                scalar2=None, op0=mybir.AluOpType.is_equal)
        nc.vector.tensor_scalar(out=identf[:], in0=it[:], scalar1=0,
                                scalar2=None, op0=mybir.AluOpType.is_equal)
        ones1 = consts.tile([1, 128], BF16)
        nc.vector.memset(ones1[:], 1.0)
        wqt = consts.tile([128, 2, C8], BF16)      # Wq^T, K-chunked
        wkt = consts.tile([128, 2, C8], BF16)
        rv = consts.tile([128, 2, C + 1], BF16)    # [Wv^T | 0] K-chunks
        rv2 = consts.tile([1, C + 1], BF16)        # [bv | 1] K-row
        bq_sb = consts.tile([128, 1], FP32)   # bq replicated 4x on partitions
        bk_sb = consts.tile([128, 1], FP32)
        gsb = consts.tile([128, 1], FP32)
        # k staggered: partitions 32j..32j+31 hold channels of m-tile 4g+j
        k_sb = consts.tile([128, NG, 128], BF16)
        q_sb = consts.tile([128, NQ], BF16)   # q replicated on 4 row groups
        vt = consts.tile([128, MT, C + 1], BF16)   # v^T + ones col, m-tiled
        xff = consts.tile([128, 2, NPIX], FP32)    # x fp32 (cols 0:NQ = queries)
        xb = consts.tile([128, 2, NPIX], BF16)

        nc.sync.dma_start(out=gsb[:], in_=g_d[:, :].to_broadcast([128, 1]))
        for bd, bt in ((bq_d, bq_sb), (bk_d, bk_sb)):
            for j in range(4):  # replicate bias onto all 4 row groups
                nc.sync.dma_start(out=bt[32 * j:32 * (j + 1), :],
                                  in_=bd[:, :])

        # ---- weight prep: transposes via PE ----
        for wd, wt in ((wq_d, wqt), (wk_d, wkt)):
            wf = stage.tile([C8, C], FP32, tag="wf")
            nc.sync.dma_start(out=wf[:], in_=wd[:, :])
            for kc in range(2):
                tp = ps_u.tile([128, C8], FP32, tag="ut")
                nc.tensor.transpose(tp[:], wf[:, kc * 128:(kc + 1) * 128],
                                    identf[:C8, :C8])
                nc.any.tensor_copy(wt[:, kc, :], tp[:])

        wvf_tiles = []
        for rh in range(2):
            wvf = stage.tile([128, C], FP32, tag=f"wvf{rh}")
            nc.sync.dma_start(out=wvf[:], in_=wv_d[rh * 128:(rh + 1) * 128, :])
            wvf_tiles.append(wvf)
        for kc in range(2):
            for jh in range(2):
                tp = ps_u.tile([128, 128], FP32, tag="ut")
                nc.tensor.transpose(tp[:],
                                    wvf_tiles[jh][:, kc * 128:(kc + 1) * 128],
                                    identf[:])
                nc.any.tensor_copy(rv[:, kc, jh * 128:(jh + 1) * 128], tp[:])
            nc.vector.memset(rv[:, kc, C:C + 1], 0.0)
        bvf = stage.tile([1, C], FP32, tag="bvf")
        nc.sync.dma_start(out=bvf[:], in_=bv_d[:, :])
        nc.vector.tensor_copy(rv2[:, 0:C], bvf[:])
        nc.vector.memset(rv2[:, C:C + 1], 1.0)

        # ---- x load + cast + projections, pipelined in 1024-col slices ----
        SL = 1024
        for sl in range(NPIX // SL):
            c0, c1 = sl * SL, (sl + 1) * SL
            for ch in range(2):
                nc.sync.dma_start(out=xff[:, ch, c0:c1],
                                  in_=xf_d[ch * 128:(ch + 1) * 128, c0:c1])
                nc.gpsimd.tensor_copy(xb[:, ch, c0:c1], xff[:, ch, c0:c1])
            for half in range(SL // 512):
                n0 = c0 + half * 512
                n1 = n0 + 512
                if n1 <= NQ:  # q projection, replicated to all 4 row groups
                    qp = ps_u.tile([128, 512], FP32, tag="ut")
                    for j in range(4):
                        for kc in range(2):
                            nc.tensor.matmul(qp[32 * j:32 * (j + 1), :],
                                             lhsT=wqt[:, kc, :],
                                             rhs=xb[:, kc, n0:n1],
                                             start=(kc == 0), stop=(kc == 1),
                                             tile_position=(0, 32 * j))
                    nc.vector.tensor_scalar_add(q_sb[:, n0:n1], in0=qp[:],
                                                scalar1=bq_sb[:])
                # k projection, staggered: row group j <- m-tile 4g+j
                g = n0 // 512
                kp = ps_u.tile([128, 128], FP32, tag="ut")
                for j in range(4):
                    m0 = n0 + j * 128
                    for kc in range(2):
                        nc.tensor.matmul(kp[32 * j:32 * (j + 1), :],
                                         lhsT=wkt[:, kc, :],
                                         rhs=xb[:, kc, m0:m0 + 128],
                                         start=(kc == 0), stop=(kc == 1),
                                         tile_position=(0, 32 * j))
                nc.vector.tensor_scalar_add(k_sb[:, g, :], in0=kp[:],
                                            scalar1=bk_sb[:])
            for mt in range(c0 // 128, c1 // 128):
                vp = ps_u.tile([128, C + 1], FP32, tag="ut")
                nc.tensor.matmul(vp[:], lhsT=xb[:, 0, mt * 128:(mt + 1) * 128],
                                 rhs=rv[:, 0, :], start=True, stop=False)
                nc.tensor.matmul(vp[:], lhsT=xb[:, 1, mt * 128:(mt + 1) * 128],
                                 rhs=rv[:, 1, :], start=False, stop=False)
                nc.tensor.matmul(vp[:], lhsT=ones1[:], rhs=rv2[:],
                                 start=False, stop=True)
                nc.vector.tensor_copy(vt[:, mt, :], vp[:])

        # ---- attention ----
        for nch in range(NCH):
            n0 = nch * 512
            uts = [ps_u.tile([128, C + 1], FP32, tag="ut", name=f"ut{nt}")
                   for nt in range(4)]
            pend = None  # one-group skew so PE never stalls on ACT's exp
            for g in range(NG):
                ss = []
                ets = []
                for pair in range(2):
                    s2 = ps_s.tile([128, 2, 512], FP32, tag="s",
                                   name=f"s2{pair}")
                    ss.append(s2)
                for j in range(4):
                    nc.tensor.matmul(ss[j // 2][:, j % 2, :],
                                     lhsT=k_sb[32 * j:32 * (j + 1), g, :],
                                     rhs=q_sb[32 * j:32 * (j + 1),
                                              n0:n0 + 512],
                                     start=True, stop=True,
                                     tile_position=(32 * j, 0))
                for pair in range(2):
                    et = etp.tile([128, 2, 512], BF16, tag="et",
                                  name=f"et{pair}")
                    nc.scalar.activation(et[:], ss[pair][:], func=AF.Exp)
                    ets.append(et)
                if dbg and nch == 0 and g == 0:
                    nc.sync.dma_start(out=etd_d[:, :, :], in_=ets[0][:])
                if pend is not None:
                    pg, pets = pend
                    for j in range(4):
                        for nt in range(4):
                            nc.tensor.matmul(
                                uts[nt][:],
                                lhsT=pets[j // 2][:, j % 2,
                                                  nt * 128:(nt + 1) * 128],
                                rhs=vt[:, 4 * pg + j, :],
                                start=(pg == 0 and j == 0), stop=False)
                pend = (g, ets)
            pg, pets = pend
            for j in range(4):
                for nt in range(4):
                    nc.tensor.matmul(
                        uts[nt][:],
                        lhsT=pets[j // 2][:, j % 2, nt * 128:(nt + 1) * 128],
                        rhs=vt[:, 4 * pg + j, :],
                        start=False, stop=(j == 3))

            if dbg and nch == 0:
                utc = small.tile([128, C + 1], FP32, tag="utc")
                nc.vector.tensor_copy(utc[:], uts[0][:])
                nc.sync.dma_start(out=utd_d[:, :], in_=utc[:])

            # epilogue: normalize, scale by gamma, transpose, residual, store
            ys = [yp.tile([128, 512], FP32, tag=f"y{ch}", name=f"ys{ch}")
                  for ch in range(2)]
            for nt in range(4):
                up = uts[nt]
                rz = small.tile([128, 1], FP32, tag="rz")
                nc.vector.reciprocal(rz[:], up[:, C:C + 1])
                rzg = small.tile([128, 1], FP32, tag="rzg")
                nc.vector.tensor_scalar_mul(rzg[:], in0=rz[:], scalar1=gsb[:])
                ot = small.tile([128, C], BF16, tag="ot")
                nc.vector.tensor_scalar_mul(ot[:], in0=up[:, 0:C],
                                            scalar1=rzg[:])
                for ch in range(2):
                    tp = ps_u.tile([128, 128], BF16, tag="ut")
                    nc.tensor.transpose(tp[:], ot[:, ch * 128:(ch + 1) * 128],
                                        ident[:])
                    nc.vector.tensor_add(
                        ys[ch][:, nt * 128:(nt + 1) * 128],
                        in0=xff[:, ch, nch * 512 + nt * 128:
                                nch * 512 + (nt + 1) * 128],
                        in1=tp[:])
            for ch in range(2):
                nc.sync.dma_start(
                    out=y_d[ch * 128:(ch + 1) * 128, nch * 512:(nch + 1) * 512],
                    in_=ys[ch][:])

        if dbg:
            nc.sync.dma_start(out=qd_d[:, :], in_=q_sb[:])
            nc.sync.dma_start(out=kd_d[:, :, :], in_=k_sb[:])
            nc.sync.dma_start(out=vtd_d[:, :, :], in_=vt[:])

    nc.finalize()
    return nc


def _in_maps(x, Wq, bq, Wk, bk, Wv, bv, gamma):
    x = np.ascontiguousarray(np.asarray(x, dtype=np.float32))
    common = {
        "wq": np.ascontiguousarray(np.asarray(Wq, np.float32)),
        "wk": np.ascontiguousarray(np.asarray(Wk, np.float32)),
        "wv": np.ascontiguousarray(np.asarray(Wv, np.float32)),
        "bq": np.ascontiguousarray(np.asarray(bq, np.float32).reshape(C8, 1)),
        "bk": np.ascontiguousarray(np.asarray(bk, np.float32).reshape(C8, 1)),
        "bv": np.ascontiguousarray(np.asarray(bv, np.float32).reshape(1, C)),
        "gamma": np.ascontiguousarray(
            np.asarray(gamma, np.float32).reshape(1, 1)),
    }
    maps = []
    for core in range(NCORES):
        b, h = divmod(core, 2)
        xf = x[b].reshape(C, NPIX)
        if h == 0:
            xr = xf
        else:  # rotate so this core's queries are columns 0..NQ-1
            xr = np.concatenate([xf[:, NQ:], xf[:, :NQ]], axis=1)
        maps.append({"xf": np.ascontiguousarray(xr), **common})
    return maps


def _run(in_maps, trace=False):
    from concourse.bass_utils import run_bass_kernel_spmd
    if "nc" not in _cache:
        _cache["nc"] = _build()
    return run_bass_kernel_spmd(_cache["nc"], in_maps,
                                core_ids=list(range(NCORES)), trace=trace)


def kernel(x, temb=None, Wq=None, bq=None, Wk=None, bk=None, Wv=None,
           bv=None, gamma=None, **_unused):
    res = _run(_in_maps(x, Wq, bq, Wk, bk, Wv, bv, gamma))
    y = np.empty((B, C, 64, 64), np.float32)
    yf = y.reshape(B, C, NPIX)
    for core in range(NCORES):
        b, h = divmod(core, 2)
        yf[b, :, h * NQ:(h + 1) * NQ] = res.results[core]["y"]
    return (y, y)


# revision 25
# speedup vs baseline: 18.2995x; 18.2995x over previous
"""Trainium2 Bass kernel for BidirectionalAttention.

Math (per batch b):
    xf = x[b].reshape(C, N)                    # C=256, N=4096
    q = Wq @ xf + bq ; k = Wk @ xf + bk        # [32, N]
    v = Wv @ xf + bv                           # [256, N]
    A = softmax_m(q^T k)                       # softmax over keys m
    out = v @ A^T ; y = x + gamma * out        # returned twice

Sharding: 8 cores = (batch b = core//2) x (query-half = core%2).
Attention is permutation-invariant over keys, so the host rotates each
core's image so its query half is always columns 0..2047 — one program
serves all cores.

On-core layout: scores are computed transposed (S^T[m, n]) so exp(S^T)
already has the contraction dim (m) on partitions for the second matmul
U^T[n, c] = sum_m E^T[m, n] * vT[m, c].  vT carries an appended ones
column so the same matmul chain yields the softmax denominator Z[n] for
free, and an appended K-row carries bv.  Normalization + gamma are a
per-partition scale on U^T; PE transposes back to [c, n] for the
residual add.  Score matmuls (K=32) are packed 4-up into the PE array
via tile_position row groups; exp runs over 2-bank PSUM tiles (FD=1024)
to halve ACT instruction overhead.
"""

import numpy as np

C = 256
C8 = 32
NPIX = 4096     # 64*64
NQ = 2048       # queries per core
B = 4
NCORES = 8
MT = NPIX // 128   # 32 key tiles
NCH = NQ // 512    # 4 query chunks per core
NG = MT // 4       # 8 groups of 4 key tiles

_cache = {}


def _build(dbg=False, stop_after=None):
    # stop_after: None | "io" | "setup" | "scores"  (perf bisection ladder)
    import concourse.bacc as bacc
    import concourse.tile as tile
    from concourse import mybir
    from contextlib import ExitStack

    FP32 = mybir.dt.float32
    BF16 = mybir.dt.bfloat16
    AF = mybir.ActivationFunctionType

    nc = bacc.Bacc("TRN2", target_bir_lowering=False, debug=False)

    xf_d = nc.dram_tensor("xf", [C, NPIX], FP32, kind="ExternalInput")
    wq_d = nc.dram_tensor("wq", [C8, C], FP32, kind="ExternalInput")
    wk_d = nc.dram_tensor("wk", [C8, C], FP32, kind="ExternalInput")
    wv_d = nc.dram_tensor("wv", [C, C], FP32, kind="ExternalInput")
    bq_d = nc.dram_tensor("bq", [C8, 1], FP32, kind="ExternalInput")
    bk_d = nc.dram_tensor("bk", [C8, 1], FP32, kind="ExternalInput")
    bv_d = nc.dram_tensor("bv", [1, C], FP32, kind="ExternalInput")
    g_d = nc.dram_tensor("gamma", [1, 1], FP32, kind="ExternalInput")
    y_d = nc.dram_tensor("y", [C, NQ], FP32, kind="ExternalOutput")
    if dbg:
        qd_d = nc.dram_tensor("qdump", [128, NQ], BF16, kind="ExternalOutput")
        kd_d = nc.dram_tensor("kdump", [128, NG, 128], BF16,
                              kind="ExternalOutput")
        vtd_d = nc.dram_tensor("vtdump", [128, MT, C + 1], BF16,
                               kind="ExternalOutput")
        etd_d = nc.dram_tensor("etdump", [128, 2, 512], BF16,
                               kind="ExternalOutput")
        utd_d = nc.dram_tensor("utdump", [128, C + 1], FP32,
                               kind="ExternalOutput")

    with tile.TileContext(nc) as tc, ExitStack() as ctx:
        consts = ctx.enter_context(tc.tile_pool(name="consts", bufs=1))
        stage = ctx.enter_context(tc.tile_pool(name="stage", bufs=2))
        etp = ctx.enter_context(tc.tile_pool(name="et", bufs=4))
        yp = ctx.enter_context(tc.tile_pool(name="yp", bufs=2))
        small = ctx.enter_context(tc.tile_pool(name="small", bufs=8))
        # PSUM: tag "s" = 2 slots x 2 banks (scores), tag "ut" = 4 slots x
        # 1 bank (U^T accumulators + all setup/epilogue psums) = 8 banks.
        ps_s = ctx.enter_context(tc.tile_pool(name="ps_s", bufs=2, space="PSUM"))
        ps_u = ctx.enter_context(tc.tile_pool(name="ps_u", bufs=4, space="PSUM"))

        # ---- persistent SBUF tensors ----
        ident = consts.tile([128, 128], BF16)
        identf = consts.tile([128, 128], FP32)
        it = consts.tile([128, 128], mybir.dt.int32)
        nc.gpsimd.iota(it[:], pattern=[[-1, 128]], base=0, channel_multiplier=1)
        nc.vector.tensor_scalar(out=ident[:], in0=it[:], scalar1=0,
                                scalar2=None, op0=mybir.AluOpType.is_equal)
        nc.vector.tensor_scalar(out=identf[:], in0=it[:], scalar1=0,
                                scalar2=None, op0=mybir.AluOpType.is_equal)
        ones1 = consts.tile([1, 128], BF16)
        nc.vector.memset(ones1[:], 1.0)
        wqt = consts.tile([128, 2, C8], BF16)      # Wq^T, K-chunked
        wkt = consts.tile([128, 2, C8], BF16)
        rv = consts.tile([128, 2, C + 1], BF16)    # [Wv^T | 0] K-chunks
        rv2 = consts.tile([1, C + 1], BF16)        # [bv | 1] K-row
        bq_sb = consts.tile([128, 1], FP32)   # bq replicated 4x on partitions
        bk_sb = consts.tile([128, 1], FP32)
        gsb = consts.tile([128, 1], FP32)
        # k staggered: partitions 32j..32j+31 hold channels of m-tile 4g+j
        k_sb = consts.tile([128, NG, 128], BF16)
        q_sb = consts.tile([128, NQ], BF16)   # q replicated on 4 row groups
        vt = consts.tile([128, MT, C + 1], BF16)   # v^T + ones col, m-tiled
        xff = consts.tile([128, 2, NPIX], FP32)    # x fp32 (cols 0:NQ = queries)
        xb = consts.tile([128, 2, NPIX], BF16)

        nc.sync.dma_start(out=gsb[:], in_=g_d[:, :].to_broadcast([128, 1]))
        for bd, bt in ((bq_d, bq_sb), (bk_d, bk_sb)):
            for j in range(4):  # replicate bias onto all 4 row groups
                nc.sync.dma_start(out=bt[32 * j:32 * (j + 1), :],
                                  in_=bd[:, :])

        # ---- weight prep: transposes via PE ----
        for wd, wt in ((wq_d, wqt), (wk_d, wkt)):
            wf = stage.tile([C8, C], FP32, tag="wf")
            nc.sync.dma_start(out=wf[:], in_=wd[:, :])
            for kc in range(2):
                tp = ps_u.tile([128, C8], FP32, tag="ut")
                nc.tensor.transpose(tp[:], wf[:, kc * 128:(kc + 1) * 128],
                                    identf[:C8, :C8])
                nc.any.tensor_copy(wt[:, kc, :], tp[:])

        wvf_tiles = []
        for rh in range(2):
            wvf = stage.tile([128, C], FP32, tag=f"wvf{rh}")
            nc.sync.dma_start(out=wvf[:], in_=wv_d[rh * 128:(rh + 1) * 128, :])
            wvf_tiles.append(wvf)
        for kc in range(2):
            for jh in range(2):
                tp = ps_u.tile([128, 128], FP32, tag="ut")
                nc.tensor.transpose(tp[:],
                                    wvf_tiles[jh][:, kc * 128:(kc + 1) * 128],
                                    identf[:])
                nc.any.tensor_copy(rv[:, kc, jh * 128:(jh + 1) * 128], tp[:])
            nc.vector.memset(rv[:, kc, C:C + 1], 0.0)
        bvf = stage.tile([1, C], FP32, tag="bvf")
        nc.sync.dma_start(out=bvf[:], in_=bv_d[:, :])
        nc.vector.tensor_copy(rv2[:, 0:C], bvf[:])
        nc.vector.memset(rv2[:, C:C + 1], 1.0)

        # ---- x load + cast + projections, pipelined in 1024-col slices ----
        SL = 1024
        for sl in range(NPIX // SL):
            c0, c1 = sl * SL, (sl + 1) * SL
            for ch in range(2):
                nc.sync.dma_start(out=xff[:, ch, c0:c1],
                                  in_=xf_d[ch * 128:(ch + 1) * 128, c0:c1])
                nc.gpsimd.tensor_copy(xb[:, ch, c0:c1], xff[:, ch, c0:c1])
            if stop_after == "io":
                continue
            for half in range(SL // 512):
                n0 = c0 + half * 512
                n1 = n0 + 512
                if n1 <= NQ:  # q projection, replicated to all 4 row groups
                    qp = ps_u.tile([128, 512], FP32, tag="ut")
                    for j in range(4):
                        for kc in range(2):
                            nc.tensor.matmul(qp[32 * j:32 * (j + 1), :],
                                             lhsT=wqt[:, kc, :],
                                             rhs=xb[:, kc, n0:n1],
                                             start=(kc == 0), stop=(kc == 1),
                                             tile_position=(0, 32 * j))
                    nc.vector.tensor_scalar_add(q_sb[:, n0:n1], in0=qp[:],
                                                scalar1=bq_sb[:])
                # k projection, staggered: row group j <- m-tile 4g+j
                g = n0 // 512
                kp = ps_u.tile([128, 128], FP32, tag="ut")
                for j in range(4):
                    m0 = n0 + j * 128
                    for kc in range(2):
                        nc.tensor.matmul(kp[32 * j:32 * (j + 1), :],
                                         lhsT=wkt[:, kc, :],
                                         rhs=xb[:, kc, m0:m0 + 128],
                                         start=(kc == 0), stop=(kc == 1),
                                         tile_position=(0, 32 * j))
                nc.vector.tensor_scalar_add(k_sb[:, g, :], in0=kp[:],
                                            scalar1=bk_sb[:])
            for mt in range(c0 // 128, c1 // 128):
                vp = ps_u.tile([128, C + 1], FP32, tag="ut")
                nc.tensor.matmul(vp[:], lhsT=xb[:, 0, mt * 128:(mt + 1) * 128],
                                 rhs=rv[:, 0, :], start=True, stop=False)
                nc.tensor.matmul(vp[:], lhsT=xb[:, 1, mt * 128:(mt + 1) * 128],
                                 rhs=rv[:, 1, :], start=False, stop=False)
                nc.tensor.matmul(vp[:], lhsT=ones1[:], rhs=rv2[:],
                                 start=False, stop=True)
                nc.vector.tensor_copy(vt[:, mt, :], vp[:])

        # ---- attention ----
        for nch in range(NCH if stop_after in (None, "scores") else 0):
            n0 = nch * 512
            if stop_after is None:
                uts = [ps_u.tile([128, C + 1], FP32, tag="ut", name=f"ut{nt}")
                       for nt in range(4)]
            pend = None  # one-group skew so PE never stalls on ACT's exp
            for g in range(NG):
                ss = []
                ets = []
                for pair in range(2):
                    s2 = ps_s.tile([128, 2, 512], FP32, tag="s",
                                   name=f"s2{pair}")
                    ss.append(s2)
                for j in range(4):
                    nc.tensor.matmul(ss[j // 2][:, j % 2, :],
                                     lhsT=k_sb[32 * j:32 * (j + 1), g, :],
                                     rhs=q_sb[32 * j:32 * (j + 1),
                                              n0:n0 + 512],
                                     start=True, stop=True,
                                     tile_position=(32 * j, 0))
                for pair in range(2):
                    et = etp.tile([128, 2, 512], BF16, tag="et",
                                  name=f"et{pair}")
                    nc.scalar.activation(et[:], ss[pair][:], func=AF.Exp)
                    ets.append(et)
                if dbg and nch == 0 and g == 0:
                    nc.sync.dma_start(out=etd_d[:, :, :], in_=ets[0][:])
                if stop_after is None and pend is not None:
                    pg, pets = pend
                    for j in range(4):
                        for nt in range(4):
                            nc.tensor.matmul(
                                uts[nt][:],
                                lhsT=pets[j // 2][:, j % 2,
                                                  nt * 128:(nt + 1) * 128],
                                rhs=vt[:, 4 * pg + j, :],
                                start=(pg == 0 and j == 0), stop=False)
                pend = (g, ets)
            if stop_after is not None:
                continue
            pg, pets = pend
            for j in range(4):
                for nt in range(4):
                    nc.tensor.matmul(
                        uts[nt][:],
                        lhsT=pets[j // 2][:, j % 2, nt * 128:(nt + 1) * 128],
                        rhs=vt[:, 4 * pg + j, :],
                        start=False, stop=(j == 3))

            if dbg and nch == 0:
                utc = small.tile([128, C + 1], FP32, tag="utc")
                nc.vector.tensor_copy(utc[:], uts[0][:])
                nc.sync.dma_start(out=utd_d[:, :], in_=utc[:])

            # epilogue: normalize, scale by gamma, transpose, residual, store
            ys = [yp.tile([128, 512], FP32, tag=f"y{ch}", name=f"ys{ch}")
                  for ch in range(2)]
            for nt in range(4):
                up = uts[nt]
                rz = small.tile([128, 1], FP32, tag="rz")
                nc.vector.reciprocal(rz[:], up[:, C:C + 1])
                rzg = small.tile([128, 1], FP32, tag="rzg")
                nc.vector.tensor_scalar_mul(rzg[:], in0=rz[:], scalar1=gsb[:])
                ot = small.tile([128, C], BF16, tag="ot")
                nc.vector.tensor_scalar_mul(ot[:], in0=up[:, 0:C],
                                            scalar1=rzg[:])
                for ch in range(2):
                    tp = ps_u.tile([128, 128], BF16, tag="ut")
                    nc.tensor.transpose(tp[:], ot[:, ch * 128:(ch + 1) * 128],
                                        ident[:])
                    nc.vector.tensor_add(
                        ys[ch][:, nt * 128:(nt + 1) * 128],
                        in0=xff[:, ch, nch * 512 + nt * 128:
                                nch * 512 + (nt + 1) * 128],
                        in1=tp[:])
            for ch in range(2):
                nc.sync.dma_start(
                    out=y_d[ch * 128:(ch + 1) * 128, nch * 512:(nch + 1) * 512],
                    in_=ys[ch][:])

        if stop_after is not None:  # stub output so the NEFF still writes y
            for nch in range(NCH):
                for ch in range(2):
                    yst = yp.tile([128, 512], FP32, tag=f"y{ch}",
                                  name=f"yst{ch}")
                    nc.vector.tensor_copy(
                        yst[:], xff[:, ch, nch * 512:(nch + 1) * 512])
                    nc.sync.dma_start(
                        out=y_d[ch * 128:(ch + 1) * 128,
                                nch * 512:(nch + 1) * 512],
                        in_=yst[:])

        if dbg:
            nc.sync.dma_start(out=qd_d[:, :], in_=q_sb[:])
            nc.sync.dma_start(out=kd_d[:, :, :], in_=k_sb[:])
            nc.sync.dma_start(out=vtd_d[:, :, :], in_=vt[:])

    nc.finalize()
    return nc


def _in_maps(x, Wq, bq, Wk, bk, Wv, bv, gamma):
    x = np.ascontiguousarray(np.asarray(x, dtype=np.float32))
    common = {
        "wq": np.ascontiguousarray(np.asarray(Wq, np.float32)),
        "wk": np.ascontiguousarray(np.asarray(Wk, np.float32)),
        "wv": np.ascontiguousarray(np.asarray(Wv, np.float32)),
        "bq": np.ascontiguousarray(np.asarray(bq, np.float32).reshape(C8, 1)),
        "bk": np.ascontiguousarray(np.asarray(bk, np.float32).reshape(C8, 1)),
        "bv": np.ascontiguousarray(np.asarray(bv, np.float32).reshape(1, C)),
        "gamma": np.ascontiguousarray(
            np.asarray(gamma, np.float32).reshape(1, 1)),
    }
    maps = []
    for core in range(NCORES):
        b, h = divmod(core, 2)
        xf = x[b].reshape(C, NPIX)
        if h == 0:
            xr = xf
        else:  # rotate so this core's queries are columns 0..NQ-1
            xr = np.concatenate([xf[:, NQ:], xf[:, :NQ]], axis=1)
        maps.append({"xf": np.ascontiguousarray(xr), **common})
    return maps


def _run(in_maps, trace=False):
    from concourse.bass_utils import run_bass_kernel_spmd
    if "nc" not in _cache:
        _cache["nc"] = _build()
    return run_bass_kernel_spmd(_cache["nc"], in_maps,
                                core_ids=list(range(NCORES)), trace=trace)


def kernel(x, temb=None, Wq=None, bq=None, Wk=None, bk=None, Wv=None,
           bv=None, gamma=None, **_unused):
    res = _run(_in_maps(x, Wq, bq, Wk, bk, Wv, bv, gamma))
    y = np.empty((B, C, 64, 64), np.float32)
    yf = y.reshape(B, C, NPIX)
    for core in range(NCORES):
        b, h = divmod(core, 2)
        yf[b, :, h * NQ:(h + 1) * NQ] = res.results[core]["y"]
    return (y, y)
